# revision 25
# baseline (speedup 1.0000x reference)
"""GCEncoder (RGCN basis-decomposition conv + mean aggregation + Dense/BN/ReLU)
as a Bass/Tile kernel on 8 Trainium2 NeuronCores.

Math (reference):
  W[r]  = sum_b comp[r,b] * basis[b]                    [R, N, H0]
  h[r]  = x @ W[r]                                      [R, N, H0]
  agg[d] = sum_r (1/cnt[d,r]) * sum_{e: dst=d, type=r} h[r, src_e]
  feats = agg + x @ root + bias
  z     = feats @ fc_w.T ; per-row batchnorm over H1 + gamma/beta + relu
  out   = (z[:U], z[U:]) stacked -> [2, U, H1]

Device strategy (per core c of 8, 512 node-rows each):
  Phase A (bf16): h_c = x[rows] @ Wall, Wall = [W[0..4] | root], processed in
           n-block groups [r0,r1][r2,r3][r4][root] so each relation's
           AllGather (fp8, 512-padded columns) launches as early as possible
           and AG4 completes well before phase B needs it.  Gathered h lands
           in a persistent SBUF tile (hres) so phase B reads it with no
           further HBM traffic.
  Phase B (fp8 DoubleRow, m-outer): per-relation segment SUMS via dense
           matmul with the EXACT integer edge-count matrix in fp8 (k=256 per
           DR matmul), then the fp32 1/cnt mean normalization is fused into
           the PSUM drain (scalar_tensor_tensor).  m-passes defer relation 4
           to absorb AG4 latency, and phase C for each finished m-block is
           interleaved into the following pass so only the last block's
           BN chain trails the final matmul.
  Phase C (bf16): feats = agg + root_part + bias; PE-transpose; fc matmul;
           per-row BN (bn_stats/bn_aggr) + gamma/beta + ReLU.
"""
import numpy as np
import ml_dtypes

import concourse.bacc as bacc
import concourse.mybir as mybir
import concourse.tile as tile
from concourse.bass_utils import run_bass_kernel_spmd
from concourse.masks import make_identity

P = 128
NCORES = 8
N = 4096          # nodes
U = 2048          # users
R = 5             # relations
H0 = 500
H0P = 512         # padded h columns (fp8 DoubleRow stream + DMA alignment)
H1 = 75
EPS = 1e-5

NL = N // NCORES              # 512 node rows per core
KB_A = N // P                 # 32 contraction tiles, phase A
MB = NL // P                  # 4 M-tiles per core
QB = 4                        # H0 chunks for transpose/fc
QS = H0 // QB                 # 125

# phase A n-block groups (n = 0..4 relations, 5 = root); singletons so each
# relation's AllGather launches as early as possible (per-AG overhead ~33us
# serializes on the CC engine — the chain must start by ~1/6 of phase A)
GROUPS = [[0], [1], [2], [3], [4], [5]]
# phase B (m, relations) passes; r4 deferred as long as possible to absorb
# the AllGather-chain latency (AG4 lands ~35us after phase A ends)
PASSES = [(0, [0, 1, 2, 3]), (1, [0, 1, 2, 3]), (2, [0, 1, 2, 3]),
          (0, [4]), (1, [4]), (2, [4]), (3, [0, 1, 2, 3, 4])]
# phase C for block m is emitted after PASSES index CPOS[m] (m=3 at the end)
CPOS = {4: 0, 5: 1, 6: 2}

F32 = mybir.dt.float32
BF16 = mybir.dt.bfloat16
FP8 = mybir.dt.float8e4
NP_FP8 = ml_dtypes.float8_e4m3

# test hooks
TRACE = False
LAST_RESULTS = None
_NC_CACHE = None


def _bf16(a: np.ndarray) -> np.ndarray:
    return np.ascontiguousarray(a).astype(ml_dtypes.bfloat16)


def _build():
    nc = bacc.Bacc("TRN2", target_bir_lowering=False, debug=False,
                   num_devices=NCORES)

    # host-swizzled inputs; layouts noted as [partition, free...]
    # x4[p, kb*NL + s] = x[coreRow s][i = kb*128+p]
    x4_d = nc.dram_tensor("x4", [P, KB_A * NL], BF16, kind="ExternalInput")
    # w4: per phase-A group g of n-blocks: [kb, j(n in group), H0] blocks
    # w4[p, goff[g] + (kb*len(g) + j)*H0 + col] = Wall[kb*128+p, (g[j])*500+col]
    w4_d = nc.dram_tensor("w4", [P, KB_A * 6 * H0], BF16,
                          kind="ExternalInput")
    # a4: exact edge counts, fp8, DoubleRow layout, m-major
    # a4[p, ((((m*R+r)*8+cb)*2+t)*2+o)*128 + mh]
    #   = count[r, src=cb*512+(2t+o)*128+p, dst=core*512+m*128+mh]
    a4_d = nc.dram_tensor("a4", [P, MB * R * NCORES * 2 * 2 * P], FP8,
                          kind="ExternalInput")
    # cinv[p, m*R + r] = 1/max(cnt[core*512+m*128+p, r], 1)
    cinv_d = nc.dram_tensor("cinv", [P, MB * R], F32, kind="ExternalInput")
    fcwt_d = nc.dram_tensor("fcwt", [H0, H1], BF16, kind="ExternalInput")
    biasb_d = nc.dram_tensor("biasb", [P, H0], BF16, kind="ExternalInput")
    gamma_d = nc.dram_tensor("gamma", [P, MB], F32, kind="ExternalInput")
    beta_d = nc.dram_tensor("beta", [P, MB], F32, kind="ExternalInput")
    out_d = nc.dram_tensor("out", [NL, H1], F32, kind="ExternalOutput")

    goff = []
    off = 0
    for g in GROUPS:
        goff.append(off)
        off += KB_A * len(g) * H0

    with tile.TileContext(nc) as tc:
        with (
            tc.tile_pool(name="big", bufs=1) as big,
            tc.tile_pool(name="slab", bufs=4) as slabp,
            tc.tile_pool(name="io", bufs=4) as iop,
            tc.tile_pool(name="bstream", bufs=3) as bsp,
            tc.tile_pool(name="persist", bufs=4) as pp,
            tc.tile_pool(name="bn", bufs=4) as bnp,
            tc.tile_pool(name="ps", bufs=8, space="PSUM") as psp,
            tc.tile_pool(name="dram", bufs=1, space="DRAM") as dramp,
        ):
            # ---------------- input streaming setup -----------------------
            # first slab/x chunks are tiny so the first matmul starts early
            XCH = [1, 3] + [4] * 7
            pre_slab = slabp.tile([P, 1, 1, H0], BF16, tag="slab",
                                  name="slab00")
            nc.sync.dma_start(out=pre_slab, in_=w4_d[:, :H0])
            xt_sb = big.tile([P, KB_A, NL], BF16, tag="xt")
            kb0 = 0
            for ci, csz in enumerate(XCH):
                nc.gpsimd.dma_start(
                    out=xt_sb[:, kb0:kb0 + csz, :],
                    in_=x4_d[:, kb0 * NL:(kb0 + csz) * NL],
                )
                kb0 += csz

            # gathered h, resident in SBUF for all of phase B
            hres = big.tile([P, R, NCORES, MB, H0P], FP8, tag="hres")

            h_cr = [dramp.tile([P, MB * H0P], FP8, tag="h_c", name=f"h_c{r}")
                    for r in range(R)]
            h_ar = [dramp.tile([NCORES * P, MB * H0P], FP8, tag="h_a",
                               addr_space="Shared", name=f"h_a{r}")
                    for r in range(R)]

            # ---------------- Phase A: h_c = x_rows @ Wall ----------------
            rootf = [None] * MB
            for gi, nlist in enumerate(GROUPS):
                wn = len(nlist)
                ps_nm = {(n, m): psp.tile([P, H0], F32, tag="ps",
                                          name=f"psA_{n}_{m}")
                         for n in nlist for m in range(MB)}
                chunks = XCH if gi == 0 else [4] * 8
                kb0 = 0
                for ci, csz in enumerate(chunks):
                    if gi == 0 and ci == 0:
                        slab = pre_slab
                    else:
                        # alternate queues: a single engine queue can't
                        # sustain the slab stream and the PE stalls cold
                        slab = slabp.tile([P, csz, wn, H0], BF16, tag="slab")
                        base = goff[gi] + kb0 * wn * H0
                        eng = nc.sync if ci % 2 == 0 else nc.scalar
                        eng.dma_start(
                            out=slab,
                            in_=w4_d[:, base:base + csz * wn * H0],
                        )
                    for k in range(csz):
                        kb = kb0 + k
                        for m in range(MB):
                            for j in range(wn):
                                nc.tensor.matmul(
                                    ps_nm[(nlist[j], m)],
                                    xt_sb[:, kb, m * P:(m + 1) * P],
                                    slab[:, k, j, :],
                                    start=(kb == 0),
                                    stop=(kb == KB_A - 1),
                                )
                    kb0 += csz
                for n in nlist:
                    for m in range(MB):
                        if n == 5:
                            # vector only: a scalar-engine copy here would
                            # queue behind the hres4 DMA's AG4-done wait and
                            # stall phase B's PSUM-slot reuse
                            rf = pp.tile([P, H0], F32, tag="rootf",
                                         name=f"rootf_{m}")
                            nc.vector.tensor_copy(out=rf, in_=ps_nm[(n, m)])
                            rootf[m] = rf
                        else:
                            hsb = iop.tile([P, H0P], FP8, tag="hout")
                            nc.vector.memset(hsb[:, H0:H0P], 0)
                            if m % 2 == 0:
                                nc.vector.tensor_copy(
                                    out=hsb[:, :H0], in_=ps_nm[(n, m)]
                                )
                            else:
                                nc.scalar.activation(
                                    out=hsb[:, :H0], in_=ps_nm[(n, m)],
                                    func=mybir.ActivationFunctionType.Copy,
                                )
                            nc.scalar.dma_start(
                                out=h_cr[n][:, m * H0P:(m + 1) * H0P],
                                in_=hsb,
                            )
                    if n < R:
                        nc.gpsimd.collective_compute(
                            "AllGather",
                            mybir.AluOpType.bypass,
                            replica_groups=[list(range(NCORES))],
                            ins=[h_cr[n][:, :]],
                            outs=[h_ar[n][:, :]],
                        )
                        nc.gpsimd.dma_start(
                            out=hres[:, n],
                            in_=h_ar[n][:, :].rearrange(
                                "(cb p) f -> p cb f", p=P),
                        )
                if gi == 2:
                    # constants (tiny; needed from phase B onward) — loaded
                    # mid-A to keep the first microseconds of DMA light
                    cinv = big.tile([P, MB * R], F32, tag="cinv")
                    nc.scalar.dma_start(out=cinv, in_=cinv_d[:, :])
                    fcw_sb = big.tile([QS, QB, H1], BF16, tag="fcw")
                    nc.scalar.dma_start(
                        out=fcw_sb,
                        in_=fcwt_d[:, :].rearrange("(q p) j -> p q j", p=QS),
                    )
                    biasb = big.tile([P, H0], BF16, tag="bias")
                    nc.scalar.dma_start(out=biasb, in_=biasb_d[:, :])
                    gam = big.tile([P, MB], F32, tag="gam")
                    nc.scalar.dma_start(out=gam, in_=gamma_d[:, :])
                    bet = big.tile([P, MB], F32, tag="bet")
                    nc.scalar.dma_start(out=bet, in_=beta_d[:, :])
                    eps_t = big.tile([P, 1], F32, tag="eps")
                    nc.vector.memset(eps_t, EPS)
                    ident = big.tile([P, P], BF16, tag="ident")
                    make_identity(nc, ident)

            # ---------------- Phase B + C interleaved ---------------------
            aggacc = [pp.tile([P, H0P], F32, tag="aggacc", name=f"agg_{m}")
                      for m in range(MB)]
            fT = [pp.tile([P, NL], BF16, tag="fT", name=f"fT_{q}")
                  for q in range(QB)]

            def b_pass(m, rlist):
                for r in rlist:
                    # weights pre-interleaved host-side (SwInterleave) so the
                    # 256-wide DoubleRow LDWEIGHTS reads contiguously
                    aam = bsp.tile([P, NCORES, 2, P, 2], FP8, tag="aa")
                    base = (m * R + r) * NCORES * 2 * 2 * P
                    nc.sync.dma_start(
                        out=aam,
                        in_=a4_d[:, base:base + NCORES * 2 * 2 * P],
                    )
                    psB = psp.tile([P, H0P], F32, tag="ps",
                                   name=f"psB_{m}_{r}")
                    for cb in range(NCORES):
                        for t in range(2):
                            nc.tensor.matmul(
                                psB,
                                aam[:, cb, t],
                                hres[:, r, cb, 2 * t:2 * t + 2, :],
                                start=(cb == 0 and t == 0),
                                stop=(cb == NCORES - 1 and t == 1),
                                perf_mode=(
                                    mybir.MatmulPerfMode.DoubleRowSwInterleave
                                ),
                            )
                    sc = cinv[:, m * R + r:m * R + r + 1]
                    if r == 0:
                        nc.vector.tensor_scalar_mul(aggacc[m], psB, sc)
                    else:
                        nc.vector.scalar_tensor_tensor(
                            out=aggacc[m], in0=psB, scalar=sc,
                            in1=aggacc[m],
                            op0=mybir.AluOpType.mult,
                            op1=mybir.AluOpType.add,
                        )

            def c_pass(m):
                f = pp.tile([P, H0], BF16, tag="feats", name=f"feats_{m}")
                nc.vector.tensor_add(out=f, in0=aggacc[m][:, :H0],
                                     in1=rootf[m])
                nc.vector.tensor_add(out=f, in0=f, in1=biasb)
                for q in range(QB):
                    pt = psp.tile([P, P], BF16, tag="ps", name=f"pt_{m}_{q}")
                    nc.tensor.transpose(
                        pt[:QS, :], f[:, q * QS:(q + 1) * QS], ident
                    )
                    nc.vector.tensor_copy(
                        out=fT[q][:QS, m * P:(m + 1) * P], in_=pt[:QS, :]
                    )
                pz = psp.tile([P, H1], F32, tag="ps", name=f"pz_{m}")
                for q in range(QB):
                    nc.tensor.matmul(
                        pz,
                        fT[q][:QS, m * P:(m + 1) * P],
                        fcw_sb[:, q, :],
                        start=(q == 0),
                        stop=(q == QB - 1),
                    )
                stats = bnp.tile([P, 6], F32, tag="stats")
                nc.vector.bn_stats(out=stats, in_=pz)
                mv = bnp.tile([P, 2], F32, tag="mv")
                nc.vector.bn_aggr(out=mv, in_=stats)
                rstd = bnp.tile([P, 1], F32, tag="rstd")
                nc.scalar.activation(
                    out=rstd, in_=mv[:, 1:2],
                    func=mybir.ActivationFunctionType.Sqrt,
                    bias=eps_t, scale=1.0,
                )
                nc.vector.reciprocal(out=rstd, in_=rstd)
                g2 = bnp.tile([P, 1], F32, tag="g2")
                nc.vector.tensor_mul(out=g2, in0=rstd, in1=gam[:, m:m + 1])
                zt = bnp.tile([P, H1], F32, tag="zt")
                nc.vector.tensor_scalar(
                    out=zt, in0=pz,
                    scalar1=mv[:, 0:1], scalar2=g2,
                    op0=mybir.AluOpType.subtract, op1=mybir.AluOpType.mult,
                )
                nc.scalar.activation(
                    out=zt, in_=zt,
                    func=mybir.ActivationFunctionType.Relu,
                    bias=bet[:, m:m + 1], scale=1.0,
                )
                nc.scalar.dma_start(out=out_d[m * P:(m + 1) * P, :], in_=zt)

            for pi, (m, rlist) in enumerate(PASSES):
                b_pass(m, rlist)
                if pi in CPOS:
                    c_pass(CPOS[pi])
            c_pass(3)

    nc.finalize()
    return nc


def _get_nc():
    global _NC_CACHE
    if _NC_CACHE is None:
        _NC_CACHE = _build()
    return _NC_CACHE


def _prepare_in_maps(inputs) -> list[dict]:
    x = np.asarray(inputs["x"], dtype=np.float32)
    basis = np.asarray(inputs["basis"], dtype=np.float32)
    comp = np.asarray(inputs["comp"], dtype=np.float32)
    root = np.asarray(inputs["root"], dtype=np.float32)
    bias_rgcn = np.asarray(inputs["bias_rgcn"], dtype=np.float32)
    fc_w = np.asarray(inputs["fc_w"], dtype=np.float32)
    bn_gamma_u = np.asarray(inputs["bn_gamma_u"], dtype=np.float32)
    bn_beta_u = np.asarray(inputs["bn_beta_u"], dtype=np.float32)
    bn_gamma_i = np.asarray(inputs["bn_gamma_i"], dtype=np.float32)
    bn_beta_i = np.asarray(inputs["bn_beta_i"], dtype=np.float32)
    edge_index = np.asarray(inputs["edge_index"]).astype(np.int64)
    edge_type = np.asarray(inputs["edge_type"]).astype(np.int64)

    src, dst = edge_index[0], edge_index[1]
    et = edge_type

    # W[r] = sum_b comp[r,b] basis[b]; Wall = [W | root]
    W = np.tensordot(comp, basis, axes=([1], [0]))          # [R, N, H0]
    wall = np.empty((N, 6 * H0), dtype=np.float32)
    wall[:, :R * H0] = W.transpose(1, 0, 2).reshape(N, R * H0)
    wall[:, R * H0:] = root
    wall16 = _bf16(wall).reshape(KB_A, P, 6, H0)            # [kb, p, n, j]
    # per phase-A group: [p, kb, j(n), col]
    parts = []
    for g in GROUPS:
        parts.append(wall16[:, :, g, :]                     # [kb, p, wn, j]
                     .transpose(1, 0, 2, 3)                 # [p, kb, wn, j]
                     .reshape(P, KB_A * len(g) * H0))
    w4 = np.ascontiguousarray(np.concatenate(parts, axis=1))

    xT16 = _bf16(x.T)                                       # [i, s]
    x4_full = (xT16.reshape(KB_A, P, N)         # [kb, p, s]
               .transpose(1, 0, 2))             # [p, kb, s]

    # exact integer edge-count matrix, fp8 (small ints are exact), and the
    # fp32 per-(dst, relation) mean normalizer
    cnt = np.bincount(dst * R + et, minlength=N * R).astype(np.float64)
    lin = (et * N + src) * np.int64(N) + dst
    acnt = np.bincount(lin, minlength=R * N * N)
    assert acnt.max() <= 16, "edge multiplicity too large for exact fp8"
    acnt8 = acnt.astype(np.float32).astype(NP_FP8).reshape(R, N, N)
    cinv_full = (1.0 / np.maximum(cnt, 1.0)).astype(np.float32).reshape(N, R)

    fcwt = _bf16(fc_w.T)
    biasb = _bf16(np.broadcast_to(bias_rgcn, (P, H0)))
    gamma_all = np.concatenate([bn_gamma_u, bn_gamma_i])
    beta_all = np.concatenate([bn_beta_u, bn_beta_i])

    in_maps = []
    for c in range(NCORES):
        sl = slice(c * NL, (c + 1) * NL)
        # a4[p, m, r, cb, t, pos, o] = acnt8[r, cb*512+(2t+o)*128+p,
        #                                    dst-slice @ mh=127-pos]
        # (DoubleRowSwInterleave weight format: o-pairs interleaved per
        #  output column, columns stored last-first)
        a4 = np.ascontiguousarray(
            acnt8[:, :, sl]                       # [r, src, d]
            .reshape(R, NCORES, 2, 2, P, MB, P)   # [r, cb, t, o, p, m, mh]
            [..., ::-1]                           # mh reversed
            .transpose(4, 5, 0, 1, 2, 3, 6)       # [p, m, r, cb, t, o, mh']
            .swapaxes(5, 6)                       # [p, m, r, cb, t, mh', o]
            .reshape(P, MB * R * NCORES * 2 * 2 * P))
        in_maps.append({
            "x4": np.ascontiguousarray(
                x4_full[:, :, sl]).reshape(P, KB_A * NL),
            "w4": w4,
            "a4": a4,
            "cinv": np.ascontiguousarray(
                cinv_full[sl].reshape(MB, P, R)   # [m, p, r]
                .transpose(1, 0, 2)               # [p, m, r]
                .reshape(P, MB * R)),
            "fcwt": fcwt,
            "biasb": biasb,
            "gamma": np.ascontiguousarray(gamma_all[sl].reshape(MB, P).T),
            "beta": np.ascontiguousarray(beta_all[sl].reshape(MB, P).T),
        })
    return in_maps


def kernel(**inputs) -> np.ndarray:
    global LAST_RESULTS
    in_maps = _prepare_in_maps(inputs)
    nc = _get_nc()
    res = run_bass_kernel_spmd(
        nc, in_maps, core_ids=list(range(NCORES)), trace=TRACE,
    )
    LAST_RESULTS = res

    z = np.concatenate([res.results[c]["out"] for c in range(NCORES)], axis=0)
    return np.stack([z[:U], z[U:]], axis=0)


# revision 28
# speedup vs baseline: 1.0032x; 1.0032x over previous
"""GCEncoder (RGCN basis-decomposition conv + mean aggregation + Dense/BN/ReLU)
as a Bass/Tile kernel on 8 Trainium2 NeuronCores.

Math (reference):
  W[r]  = sum_b comp[r,b] * basis[b]                    [R, N, H0]
  h[r]  = x @ W[r]                                      [R, N, H0]
  agg[d] = sum_r (1/cnt[d,r]) * sum_{e: dst=d, type=r} h[r, src_e]
  feats = agg + x @ root + bias
  z     = feats @ fc_w.T ; per-row batchnorm over H1 + gamma/beta + relu
  out   = (z[:U], z[U:]) stacked -> [2, U, H1]

Device strategy (per core c of 8, 512 node-rows each):
  Phase A (bf16): h_c = x[rows] @ Wall, Wall = [W[0..4] | root], processed in
           n-block groups [r0,r1][r2,r3][r4][root] so each relation's
           AllGather (fp8, 512-padded columns) launches as early as possible
           and AG4 completes well before phase B needs it.  Gathered h lands
           in a persistent SBUF tile (hres) so phase B reads it with no
           further HBM traffic.
  Phase B (fp8 DoubleRow, m-outer): per-relation segment SUMS via dense
           matmul with the EXACT integer edge-count matrix in fp8 (k=256 per
           DR matmul), then the fp32 1/cnt mean normalization is fused into
           the PSUM drain (scalar_tensor_tensor).  m-passes defer relation 4
           to absorb AG4 latency, and phase C for each finished m-block is
           interleaved into the following pass so only the last block's
           BN chain trails the final matmul.
  Phase C (bf16): feats = agg + root_part + bias; PE-transpose; fc matmul;
           per-row BN (bn_stats/bn_aggr) + gamma/beta + ReLU.
"""
import numpy as np
import ml_dtypes

import concourse.bacc as bacc
import concourse.mybir as mybir
import concourse.tile as tile
from concourse.bass_utils import run_bass_kernel_spmd
from concourse.masks import make_identity

P = 128
NCORES = 8
N = 4096          # nodes
U = 2048          # users
R = 5             # relations
H0 = 500
H0P = 512         # padded h columns (fp8 DoubleRow stream + DMA alignment)
H1 = 75
EPS = 1e-5

NL = N // NCORES              # 512 node rows per core
KB_A = N // P                 # 32 contraction tiles, phase A
MB = NL // P                  # 4 M-tiles per core
QB = 4                        # H0 chunks for transpose/fc
QS = H0 // QB                 # 125

# phase A n-block groups (n = 0..4 relations, 5 = root); singletons so each
# relation's AllGather launches as early as possible (per-AG overhead ~33us
# serializes on the CC engine — the chain must start by ~1/6 of phase A)
GROUPS = [[0], [1], [2], [3], [4], [5]]
# phase B (m, relations) passes; r4 deferred as long as possible to absorb
# the AllGather-chain latency (AG4 lands ~35us after phase A ends)
PASSES = [(0, [0, 1, 2, 3]), (1, [0, 1, 2, 3]), (2, [0, 1, 2, 3]),
          (0, [4]), (1, [4]), (2, [4]), (3, [0, 1, 2, 3, 4])]
# phase C for block m is emitted after PASSES index CPOS[m] (m=3 at the end)
CPOS = {4: 0, 5: 1, 6: 2}

F32 = mybir.dt.float32
BF16 = mybir.dt.bfloat16
FP8 = mybir.dt.float8e4
NP_FP8 = ml_dtypes.float8_e4m3

# test hooks
TRACE = False
LAST_RESULTS = None
_NC_CACHE = None


def _bf16(a: np.ndarray) -> np.ndarray:
    return np.ascontiguousarray(a).astype(ml_dtypes.bfloat16)


def _build():
    nc = bacc.Bacc("TRN2", target_bir_lowering=False, debug=False,
                   num_devices=NCORES)

    # host-swizzled inputs; layouts noted as [partition, free...]
    # x4[p, kb*NL + s] = x[coreRow s][i = kb*128+p]
    x4_d = nc.dram_tensor("x4", [P, KB_A * NL], BF16, kind="ExternalInput")
    # w4: per phase-A group g of n-blocks: [kb, j(n in group), H0] blocks
    # w4[p, goff[g] + (kb*len(g) + j)*H0 + col] = Wall[kb*128+p, (g[j])*500+col]
    w4_d = nc.dram_tensor("w4", [P, KB_A * 6 * H0], BF16,
                          kind="ExternalInput")
    # a4: exact edge counts, fp8, DoubleRow layout, m-major
    # a4[p, ((((m*R+r)*8+cb)*2+t)*2+o)*128 + mh]
    #   = count[r, src=cb*512+(2t+o)*128+p, dst=core*512+m*128+mh]
    a4_d = nc.dram_tensor("a4", [P, MB * R * NCORES * 2 * 2 * P], FP8,
                          kind="ExternalInput")
    # cinv[p, m*R + r] = 1/max(cnt[core*512+m*128+p, r], 1)
    cinv_d = nc.dram_tensor("cinv", [P, MB * R], F32, kind="ExternalInput")
    fcwt_d = nc.dram_tensor("fcwt", [H0, H1], BF16, kind="ExternalInput")
    biasb_d = nc.dram_tensor("biasb", [P, H0], BF16, kind="ExternalInput")
    gamma_d = nc.dram_tensor("gamma", [P, MB], F32, kind="ExternalInput")
    beta_d = nc.dram_tensor("beta", [P, MB], F32, kind="ExternalInput")
    out_d = nc.dram_tensor("out", [NL, H1], F32, kind="ExternalOutput")

    goff = []
    off = 0
    for g in GROUPS:
        goff.append(off)
        off += KB_A * len(g) * H0

    with tile.TileContext(nc) as tc:
        with (
            tc.tile_pool(name="big", bufs=1) as big,
            tc.tile_pool(name="slab", bufs=3) as slabp,
            tc.tile_pool(name="io", bufs=4) as iop,
            tc.tile_pool(name="bstream", bufs=3) as bsp,
            tc.tile_pool(name="persist", bufs=4) as pp,
            tc.tile_pool(name="bn", bufs=4) as bnp,
            tc.tile_pool(name="ps", bufs=8, space="PSUM") as psp,
            tc.tile_pool(name="dram", bufs=1, space="DRAM") as dramp,
        ):
            # ---------------- input streaming setup -----------------------
            # first slab/x chunks are tiny so the first matmul starts early
            XCH = [1, 3] + [4] * 7
            pre_slab = slabp.tile([P, 1, 1, H0], BF16, tag="slab",
                                  name="slab00")
            nc.sync.dma_start(out=pre_slab, in_=w4_d[:, :H0])
            xt_sb = big.tile([P, KB_A, NL], BF16, tag="xt")
            kb0 = 0
            for ci, csz in enumerate(XCH):
                eng = nc.scalar if ci % 2 == 0 else nc.gpsimd
                eng.dma_start(
                    out=xt_sb[:, kb0:kb0 + csz, :],
                    in_=x4_d[:, kb0 * NL:(kb0 + csz) * NL],
                )
                kb0 += csz

            # gathered h, resident in SBUF for all of phase B
            hres = big.tile([P, R, NCORES, MB, H0P], FP8, tag="hres")

            h_cr = [dramp.tile([P, MB * H0P], FP8, tag="h_c", name=f"h_c{r}")
                    for r in range(R)]
            h_ar = [dramp.tile([NCORES * P, MB * H0P], FP8, tag="h_a",
                               addr_space="Shared", name=f"h_a{r}")
                    for r in range(R)]

            # ---------------- Phase A: h_c = x_rows @ Wall ----------------
            rootf = [None] * MB
            for gi, nlist in enumerate(GROUPS):
                wn = len(nlist)
                ps_nm = {(n, m): psp.tile([P, H0], F32, tag="ps",
                                          name=f"psA_{n}_{m}")
                         for n in nlist for m in range(MB)}
                chunks = XCH if gi == 0 else [8] * 4
                kb0 = 0
                for ci, csz in enumerate(chunks):
                    if gi == 0 and ci == 0:
                        slab = pre_slab
                    else:
                        slab = slabp.tile([P, csz, wn, H0], BF16, tag="slab")
                        base = goff[gi] + kb0 * wn * H0
                        nc.sync.dma_start(
                            out=slab,
                            in_=w4_d[:, base:base + csz * wn * H0],
                        )
                    for k in range(csz):
                        kb = kb0 + k
                        for m in range(MB):
                            for j in range(wn):
                                nc.tensor.matmul(
                                    ps_nm[(nlist[j], m)],
                                    xt_sb[:, kb, m * P:(m + 1) * P],
                                    slab[:, k, j, :],
                                    start=(kb == 0),
                                    stop=(kb == KB_A - 1),
                                )
                    kb0 += csz
                for n in nlist:
                    for m in range(MB):
                        if n == 5:
                            # vector only: a scalar-engine copy here would
                            # queue behind the hres4 DMA's AG4-done wait and
                            # stall phase B's PSUM-slot reuse
                            rf = pp.tile([P, H0], F32, tag="rootf",
                                         name=f"rootf_{m}")
                            nc.vector.tensor_copy(out=rf, in_=ps_nm[(n, m)])
                            rootf[m] = rf
                        else:
                            hsb = iop.tile([P, H0P], FP8, tag="hout")
                            nc.vector.memset(hsb[:, H0:H0P], 0)
                            if m % 2 == 0:
                                nc.vector.tensor_copy(
                                    out=hsb[:, :H0], in_=ps_nm[(n, m)]
                                )
                            else:
                                nc.scalar.activation(
                                    out=hsb[:, :H0], in_=ps_nm[(n, m)],
                                    func=mybir.ActivationFunctionType.Copy,
                                )
                            nc.scalar.dma_start(
                                out=h_cr[n][:, m * H0P:(m + 1) * H0P],
                                in_=hsb,
                            )
                    if n < R:
                        nc.gpsimd.collective_compute(
                            "AllGather",
                            mybir.AluOpType.bypass,
                            replica_groups=[list(range(NCORES))],
                            ins=[h_cr[n][:, :]],
                            outs=[h_ar[n][:, :]],
                        )
                        nc.gpsimd.dma_start(
                            out=hres[:, n],
                            in_=h_ar[n][:, :].rearrange(
                                "(cb p) f -> p cb f", p=P),
                        )
                if gi == 2:
                    # constants (tiny; needed from phase B onward) — loaded
                    # mid-A to keep the first microseconds of DMA light
                    cinv = big.tile([P, MB * R], F32, tag="cinv")
                    nc.scalar.dma_start(out=cinv, in_=cinv_d[:, :])
                    fcw_sb = big.tile([QS, QB, H1], BF16, tag="fcw")
                    nc.scalar.dma_start(
                        out=fcw_sb,
                        in_=fcwt_d[:, :].rearrange("(q p) j -> p q j", p=QS),
                    )
                    biasb = big.tile([P, H0], BF16, tag="bias")
                    nc.scalar.dma_start(out=biasb, in_=biasb_d[:, :])
                    gam = big.tile([P, MB], F32, tag="gam")
                    nc.scalar.dma_start(out=gam, in_=gamma_d[:, :])
                    bet = big.tile([P, MB], F32, tag="bet")
                    nc.scalar.dma_start(out=bet, in_=beta_d[:, :])
                    eps_t = big.tile([P, 1], F32, tag="eps")
                    nc.vector.memset(eps_t, EPS)
                    ident = big.tile([P, P], BF16, tag="ident")
                    make_identity(nc, ident)

            # ---------------- Phase B + C interleaved ---------------------
            aggacc = [pp.tile([P, H0P], F32, tag="aggacc", name=f"agg_{m}")
                      for m in range(MB)]
            fT = [pp.tile([P, NL], BF16, tag="fT", name=f"fT_{q}")
                  for q in range(QB)]

            def b_pass(m, rlist):
                for r in rlist:
                    # weights pre-interleaved host-side (SwInterleave) so the
                    # 256-wide DoubleRow LDWEIGHTS reads contiguously
                    aam = bsp.tile([P, NCORES, 2, P, 2], FP8, tag="aa")
                    base = (m * R + r) * NCORES * 2 * 2 * P
                    nc.sync.dma_start(
                        out=aam,
                        in_=a4_d[:, base:base + NCORES * 2 * 2 * P],
                    )
                    psB = psp.tile([P, H0P], F32, tag="ps",
                                   name=f"psB_{m}_{r}")
                    for cb in range(NCORES):
                        for t in range(2):
                            nc.tensor.matmul(
                                psB,
                                aam[:, cb, t],
                                hres[:, r, cb, 2 * t:2 * t + 2, :],
                                start=(cb == 0 and t == 0),
                                stop=(cb == NCORES - 1 and t == 1),
                                perf_mode=(
                                    mybir.MatmulPerfMode.DoubleRowSwInterleave
                                ),
                            )
                    sc = cinv[:, m * R + r:m * R + r + 1]
                    if r == 0:
                        nc.vector.tensor_scalar_mul(aggacc[m], psB, sc)
                    else:
                        nc.vector.scalar_tensor_tensor(
                            out=aggacc[m], in0=psB, scalar=sc,
                            in1=aggacc[m],
                            op0=mybir.AluOpType.mult,
                            op1=mybir.AluOpType.add,
                        )

            def c_pass(m):
                f = pp.tile([P, H0], BF16, tag="feats", name=f"feats_{m}")
                nc.vector.tensor_add(out=f, in0=aggacc[m][:, :H0],
                                     in1=rootf[m])
                nc.vector.tensor_add(out=f, in0=f, in1=biasb)
                for q in range(QB):
                    pt = psp.tile([P, P], BF16, tag="ps", name=f"pt_{m}_{q}")
                    nc.tensor.transpose(
                        pt[:QS, :], f[:, q * QS:(q + 1) * QS], ident
                    )
                    nc.vector.tensor_copy(
                        out=fT[q][:QS, m * P:(m + 1) * P], in_=pt[:QS, :]
                    )
                pz = psp.tile([P, H1], F32, tag="ps", name=f"pz_{m}")
                for q in range(QB):
                    nc.tensor.matmul(
                        pz,
                        fT[q][:QS, m * P:(m + 1) * P],
                        fcw_sb[:, q, :],
                        start=(q == 0),
                        stop=(q == QB - 1),
                    )
                stats = bnp.tile([P, 6], F32, tag="stats")
                nc.vector.bn_stats(out=stats, in_=pz)
                mv = bnp.tile([P, 2], F32, tag="mv")
                nc.vector.bn_aggr(out=mv, in_=stats)
                rstd = bnp.tile([P, 1], F32, tag="rstd")
                nc.scalar.activation(
                    out=rstd, in_=mv[:, 1:2],
                    func=mybir.ActivationFunctionType.Sqrt,
                    bias=eps_t, scale=1.0,
                )
                nc.vector.reciprocal(out=rstd, in_=rstd)
                g2 = bnp.tile([P, 1], F32, tag="g2")
                nc.vector.tensor_mul(out=g2, in0=rstd, in1=gam[:, m:m + 1])
                zt = bnp.tile([P, H1], F32, tag="zt")
                nc.vector.tensor_scalar(
                    out=zt, in0=pz,
                    scalar1=mv[:, 0:1], scalar2=g2,
                    op0=mybir.AluOpType.subtract, op1=mybir.AluOpType.mult,
                )
                nc.scalar.activation(
                    out=zt, in_=zt,
                    func=mybir.ActivationFunctionType.Relu,
                    bias=bet[:, m:m + 1], scale=1.0,
                )
                nc.scalar.dma_start(out=out_d[m * P:(m + 1) * P, :], in_=zt)

            for pi, (m, rlist) in enumerate(PASSES):
                b_pass(m, rlist)
                if pi in CPOS:
                    c_pass(CPOS[pi])
            c_pass(3)

    nc.finalize()
    return nc


def _get_nc():
    global _NC_CACHE
    if _NC_CACHE is None:
        _NC_CACHE = _build()
    return _NC_CACHE


def _prepare_in_maps(inputs) -> list[dict]:
    x = np.asarray(inputs["x"], dtype=np.float32)
    basis = np.asarray(inputs["basis"], dtype=np.float32)
    comp = np.asarray(inputs["comp"], dtype=np.float32)
    root = np.asarray(inputs["root"], dtype=np.float32)
    bias_rgcn = np.asarray(inputs["bias_rgcn"], dtype=np.float32)
    fc_w = np.asarray(inputs["fc_w"], dtype=np.float32)
    bn_gamma_u = np.asarray(inputs["bn_gamma_u"], dtype=np.float32)
    bn_beta_u = np.asarray(inputs["bn_beta_u"], dtype=np.float32)
    bn_gamma_i = np.asarray(inputs["bn_gamma_i"], dtype=np.float32)
    bn_beta_i = np.asarray(inputs["bn_beta_i"], dtype=np.float32)
    edge_index = np.asarray(inputs["edge_index"]).astype(np.int64)
    edge_type = np.asarray(inputs["edge_type"]).astype(np.int64)

    src, dst = edge_index[0], edge_index[1]
    et = edge_type

    # W[r] = sum_b comp[r,b] basis[b]; Wall = [W | root]
    W = np.tensordot(comp, basis, axes=([1], [0]))          # [R, N, H0]
    wall = np.empty((N, 6 * H0), dtype=np.float32)
    wall[:, :R * H0] = W.transpose(1, 0, 2).reshape(N, R * H0)
    wall[:, R * H0:] = root
    wall16 = _bf16(wall).reshape(KB_A, P, 6, H0)            # [kb, p, n, j]
    # per phase-A group: [p, kb, j(n), col]
    parts = []
    for g in GROUPS:
        parts.append(wall16[:, :, g, :]                     # [kb, p, wn, j]
                     .transpose(1, 0, 2, 3)                 # [p, kb, wn, j]
                     .reshape(P, KB_A * len(g) * H0))
    w4 = np.ascontiguousarray(np.concatenate(parts, axis=1))

    xT16 = _bf16(x.T)                                       # [i, s]
    x4_full = (xT16.reshape(KB_A, P, N)         # [kb, p, s]
               .transpose(1, 0, 2))             # [p, kb, s]

    # exact integer edge-count matrix, fp8 (small ints are exact), and the
    # fp32 per-(dst, relation) mean normalizer
    cnt = np.bincount(dst * R + et, minlength=N * R).astype(np.float64)
    lin = (et * N + src) * np.int64(N) + dst
    acnt = np.bincount(lin, minlength=R * N * N)
    assert acnt.max() <= 16, "edge multiplicity too large for exact fp8"
    acnt8 = acnt.astype(np.float32).astype(NP_FP8).reshape(R, N, N)
    cinv_full = (1.0 / np.maximum(cnt, 1.0)).astype(np.float32).reshape(N, R)

    fcwt = _bf16(fc_w.T)
    biasb = _bf16(np.broadcast_to(bias_rgcn, (P, H0)))
    gamma_all = np.concatenate([bn_gamma_u, bn_gamma_i])
    beta_all = np.concatenate([bn_beta_u, bn_beta_i])

    in_maps = []
    for c in range(NCORES):
        sl = slice(c * NL, (c + 1) * NL)
        # a4[p, m, r, cb, t, pos, o] = acnt8[r, cb*512+(2t+o)*128+p,
        #                                    dst-slice @ mh=127-pos]
        # (DoubleRowSwInterleave weight format: o-pairs interleaved per
        #  output column, columns stored last-first)
        a4 = np.ascontiguousarray(
            acnt8[:, :, sl]                       # [r, src, d]
            .reshape(R, NCORES, 2, 2, P, MB, P)   # [r, cb, t, o, p, m, mh]
            [..., ::-1]                           # mh reversed
            .transpose(4, 5, 0, 1, 2, 3, 6)       # [p, m, r, cb, t, o, mh']
            .swapaxes(5, 6)                       # [p, m, r, cb, t, mh', o]
            .reshape(P, MB * R * NCORES * 2 * 2 * P))
        in_maps.append({
            "x4": np.ascontiguousarray(
                x4_full[:, :, sl]).reshape(P, KB_A * NL),
            "w4": w4,
            "a4": a4,
            "cinv": np.ascontiguousarray(
                cinv_full[sl].reshape(MB, P, R)   # [m, p, r]
                .transpose(1, 0, 2)               # [p, m, r]
                .reshape(P, MB * R)),
            "fcwt": fcwt,
            "biasb": biasb,
            "gamma": np.ascontiguousarray(gamma_all[sl].reshape(MB, P).T),
            "beta": np.ascontiguousarray(beta_all[sl].reshape(MB, P).T),
        })
    return in_maps


def kernel(**inputs) -> np.ndarray:
    global LAST_RESULTS
    in_maps = _prepare_in_maps(inputs)
    nc = _get_nc()
    res = run_bass_kernel_spmd(
        nc, in_maps, core_ids=list(range(NCORES)), trace=TRACE,
    )
    LAST_RESULTS = res

    z = np.concatenate([res.results[c]["out"] for c in range(NCORES)], axis=0)
    return np.stack([z[:U], z[U:]], axis=0)
